# revision 4
# baseline (speedup 1.0000x reference)
"""GIN message-passing encoder (3 layers) on 8 Trainium2 NeuronCores.

Problem: x_{l+1} = relu(BN(relu((x + agg(x)) @ W1 + b1) @ W2 + b2)),
agg[b, d] = sum over edges (s -> d) of x[b, s]; output = stack of the 3
layer outputs, shape [3, 16, 1024, 256].

Strategy
--------
- Data parallel over batch: B=16 split as 2 batch elements per core.
- The scatter-add is a dense matmul against a host-built (N x N) matrix
  Bm[s, d] = I[s, d] + multiplicity(edge s -> d); the +x of GIN(eps=0)
  is the identity fold. Runs in fp8e4 DoubleRow (K=256/instruction).
- Eval-mode BatchNorm is folded into W2/b2 on the host; MLP matmuls in
  bf16.
- The PE stream is scheduled gapless: per layer the order is
  S1(b0) S1(b1) S2(b0) S2(b1) S3(b0) S3(b1), so each stage's producers
  (DVE/ACT casts and relus) complete during unrelated PE work. The PE
  p-state doubles its clock after ~3us of continuous work, so PE gaps
  cost double.
- Elementwise work is spread over three engines so none gates the PE:
  ACT does the ft0 m0-cast + half0 step2-relu + even-tp y-relu; DVE
  does the ft1/half1/odd-tp counterparts; GPSIMD (otherwise idle,
  no PSUM port) re-quantizes y (SBUF f32r) to the next layer's fp8 x.
- PSUM: step1 uses 2x [P,1024] bufs (4 banks); all step2/step3 groups
  share one 4-buf [P,512] pool (4 banks) so bank reuse never stalls.
- b2 bias enters step3's PSUM via a ones-matmul (lhsT=ones[128,128],
  rhs holds b2' pair on partition 0) in one N=512 instruction.
- Inputs are host-preswizzled; loads are spread across the scalar and
  vector queues in first-use order, stores ride the sync queue as
  [P,4,F] half-batches.
"""

import os

import numpy as np

BN_EPS = 1e-5

B, N, F = 16, 1024, 256
L = 3
NCORES = 8
BPC = B // NCORES  # batch elements per core
P = 128
NT = N // P   # 8 node tiles
FT = F // P   # 2 feature tiles
KK = N // 256  # 4 double-chunks of the contraction dim (DoubleRow K=256)
HALF = 512    # moving free-dim chunk
NH = N // HALF  # 2 halves of the node dim

_cache: dict = {}


def _build_nc():
    import concourse.bacc as bacc
    import concourse.mybir as mybir
    import concourse.tile as tile

    F32 = mybir.dt.float32
    F32R = mybir.dt.float32r
    F8 = mybir.dt.float8e4
    BF16 = mybir.dt.bfloat16
    Relu = mybir.ActivationFunctionType.Relu
    Copy = mybir.ActivationFunctionType.Copy
    Alu = mybir.AluOpType
    DR = mybir.MatmulPerfMode.DoubleRow

    nc = bacc.Bacc()

    x0hi_d = nc.dram_tensor("x0hi", [BPC, P, KK, 2, F], F8, kind="ExternalInput")
    bm_d = nc.dram_tensor("bm", [P, KK, 2, N], F8, kind="ExternalInput")
    w1_d = nc.dram_tensor("w1", [P, L, FT, F], BF16, kind="ExternalInput")
    w2_d = nc.dram_tensor("w2", [P, L, FT, F], BF16, kind="ExternalInput")
    b1_d = nc.dram_tensor("b1", [P, L * FT], F32, kind="ExternalInput")
    b2_d = nc.dram_tensor("b2", [P, L, 2 * F], BF16, kind="ExternalInput")
    ones_d = nc.dram_tensor("ones", [P, P], BF16, kind="ExternalInput")
    out_d = nc.dram_tensor("out", [L, BPC, N, F], F32R, kind="ExternalOutput")

    with tile.TileContext(nc) as tc:
        with (
            tc.tile_pool(name="const", bufs=1) as cpool,
            tc.tile_pool(name="x8", bufs=2) as xpool,
            tc.tile_pool(name="m0", bufs=2) as mpool,
            tc.tile_pool(name="h1", bufs=2) as hpool,
            tc.tile_pool(name="yt", bufs=4) as ypool,
            tc.tile_pool(name="pm0", bufs=2, space="PSUM") as pm0,
            tc.tile_pool(name="pmlp", bufs=4, space="PSUM") as pmlp,
        ):
            bm_sb = cpool.tile([P, KK, 2, N], F8)
            w1_sb = cpool.tile([P, L, FT, F], BF16)
            w2_sb = cpool.tile([P, L, FT, F], BF16)
            b1_sb = cpool.tile([P, L * FT], F32)
            b2z_sb = cpool.tile([P, L, 2 * F], BF16)
            ones_sb = cpool.tile([P, P], BF16)

            xhi = xpool.tile([P, BPC, KK, 2, F], F8, tag="xhi")

            # Input DMAs in first-use order. sync: bm (step1 rhs, kk
            # granular so MM kk0 starts after 256KB). scalar: x(b0)
            # then step-2 weights. vector: step-3 consts then x(b1).
            for kk in range(KK):
                nc.sync.dma_start(bm_sb[:, kk:kk + 1], bm_d[:, kk:kk + 1])
            nc.scalar.dma_start(xhi[:, 0, 0:1], x0hi_d[0, :, 0:1])
            nc.scalar.dma_start(xhi[:, 0, 1:4], x0hi_d[0, :, 1:4])
            nc.scalar.dma_start(w1_sb[:], w1_d[:])
            nc.scalar.dma_start(b1_sb[:], b1_d[:])
            nc.sync.dma_start(xhi[:, 1], x0hi_d[1])
            nc.sync.dma_start(ones_sb[:], ones_d[:])
            nc.sync.dma_start(w2_sb[:], w2_d[:])
            nc.sync.dma_start(b2z_sb[:], b2_d[:])

            for l in range(L):
                last = l == L - 1
                if not last:
                    nxhi = xpool.tile([P, BPC, KK, 2, F], F8, tag="xhi")

                # ---- step 1: m0T = (A + I) @ x_q, fp8 DoubleRow ----
                m0t = [None, None]
                for b in range(BPC):
                    m0t[b] = mpool.tile([P, FT, N], BF16, tag="m0t", name=f"m0t{b}")
                    for ft in range(FT):
                        ps = pm0.tile([P, NH * HALF], F32, tag="pm0")
                        for kk in range(KK):
                            for half in range(NH):
                                nc.tensor.matmul(
                                    ps[:, half * HALF:(half + 1) * HALF],
                                    xhi[:, b, kk, :, ft * P:(ft + 1) * P],
                                    bm_sb[:, kk, :,
                                          half * HALF:(half + 1) * HALF],
                                    start=(kk == 0),
                                    stop=(kk == KK - 1),
                                    perf_mode=DR,
                                )
                        # PSUM->SBUF cast split across ACT (ft0) and DVE
                        # (ft1): either alone is slower than the
                        # full-clock PE producing the next group.
                        if ft == 0:
                            nc.scalar.activation(m0t[b][:, ft, :], ps[:], Copy)
                        else:
                            nc.vector.tensor_copy(m0t[b][:, ft, :], ps[:])

                h1t = [None, None]
                ysb = [[None, None], [None, None]]
                for b in range(BPC):
                    # ---- step 2: h1T = relu(W1^T-contract @ m0T + b1) ----
                    h1t[b] = hpool.tile([P, FT, N], BF16, tag="h1t", name=f"h1t{b}")
                    for gt in range(FT):
                        pg = [pmlp.tile([P, HALF], F32, tag="mlp", name=f"pg{h}")
                              for h in range(NH)]
                        for fk in range(FT):
                            for half in range(NH):
                                nc.tensor.matmul(
                                    pg[half][:],
                                    w1_sb[:, l, fk, gt * P:(gt + 1) * P],
                                    m0t[b][:, fk,
                                           half * HALF:(half + 1) * HALF],
                                    start=(fk == 0),
                                    stop=(fk == FT - 1),
                                )
                        # relu+bias split: ACT takes half0, DVE half1
                        nc.scalar.activation(
                            h1t[b][:, gt, 0:HALF], pg[0][:], Relu,
                            bias=b1_sb[:, l * FT + gt:l * FT + gt + 1],
                        )
                        nc.vector.tensor_scalar(
                            h1t[b][:, gt, HALF:N], pg[1][:],
                            b1_sb[:, l * FT + gt:l * FT + gt + 1], 0.0,
                            op0=Alu.add, op1=Alu.max,
                        )

                for b in range(BPC):
                    # ---- step 3: y = relu(h1 @ W2' + b2') -> out + next x ----
                    for j in range(2):        # output half: nodes [512j, 512j+512)
                        ysb[b][j] = ypool.tile([P, 4, F], F32R, tag="y", name=f"y{b}{j}")
                        for t2 in range(2):   # tp = 2j + t2
                            tp = 2 * j + t2
                            ps3 = pmlp.tile([P, 2, F], F32, tag="mlp")
                            # seed b2' into PSUM via a ones-matmul
                            # (N=512 covers both node tiles), then
                            # accumulate both tiles' GEMMs.
                            nc.tensor.matmul(
                                ps3[:], ones_sb[:], b2z_sb[:, l, :],
                                start=True, stop=False, skip_group_check=True,
                            )
                            for jj in range(2):
                                nt = 2 * tp + jj
                                for gk in range(FT):
                                    nc.tensor.matmul(
                                        ps3[:, jj, :],
                                        h1t[b][:, gk, nt * P:(nt + 1) * P],
                                        w2_sb[:, l, gk, :],
                                        start=False,
                                        stop=(gk == FT - 1),
                                        skip_group_check=True,
                                    )
                            ydst = ysb[b][j][:, 2 * t2:2 * t2 + 2, :]
                            if t2 == 0:
                                nc.scalar.activation(ydst, ps3[:], Relu)
                            else:
                                nc.vector.tensor_scalar(
                                    ydst, ps3[:], 0.0, None, op0=Alu.max,
                                )
                        if last and b == BPC - 1:
                            # tail: split the final stores so the drain
                            # overlaps the remaining compute
                            for t2 in range(2):
                                nc.sync.dma_start(
                                    out_d[l, b,
                                          (4 * j + 2 * t2) * P:
                                          (4 * j + 2 * t2 + 2) * P, :]
                                    .rearrange("(t p) f -> p t f", p=P),
                                    ysb[b][j][:, 2 * t2:2 * t2 + 2, :],
                                )
                        else:
                            nc.sync.dma_start(
                                out_d[l, b, j * 4 * P:(j + 1) * 4 * P, :]
                                .rearrange("(t p) f -> p t f", p=P),
                                ysb[b][j][:],
                            )
                        if not last:
                            # next-layer fp8 x on the idle GPSIMD engine
                            # (y >= 0 already; pure quantizing copy)
                            nc.gpsimd.tensor_copy(
                                nxhi[:, b, 2 * j:2 * j + 2, :, :],
                                ysb[b][j][:],
                            )
                if not last:
                    xhi = nxhi

    nc.finalize()
    return nc


def kernel(h, edge_index, W1, b1, W2, b2, gamma, beta, run_mean, run_var):
    import ml_dtypes
    from concourse.bass_utils import run_bass_kernel_spmd

    f8 = ml_dtypes.float8_e4m3

    h = np.asarray(h, dtype=np.float32)
    edge_index = np.asarray(edge_index)
    W1 = np.asarray(W1, dtype=np.float32)
    b1 = np.asarray(b1, dtype=np.float32)
    W2 = np.asarray(W2, dtype=np.float32)
    b2 = np.asarray(b2, dtype=np.float32)
    gamma = np.asarray(gamma, dtype=np.float32)
    beta = np.asarray(beta, dtype=np.float32)
    run_mean = np.asarray(run_mean, dtype=np.float32)
    run_var = np.asarray(run_var, dtype=np.float32)

    # host-side preprocessing
    src = edge_index[0].astype(np.int64)
    dst = edge_index[1].astype(np.int64)
    bm = np.zeros((N, N), dtype=np.float32)
    np.add.at(bm, (src, dst), 1.0)
    bm[np.arange(N), np.arange(N)] += 1.0
    # fp8 exact for small integer counts; DoubleRow layout [P, KK, 2, N]
    bm8 = np.ascontiguousarray(
        bm.astype(f8).reshape(KK, 2, P, N).transpose(2, 0, 1, 3)
    )

    # x0 quantized to fp8 on the host, swizzled to [B, P, KK, 2, F]
    xhi8s = np.ascontiguousarray(
        h.astype(f8).reshape(B, KK, 2, P, F).transpose(0, 3, 1, 2, 4)
    )

    inv = (gamma / np.sqrt(run_var + BN_EPS)).astype(np.float32)      # [L, F]
    w2f = (W2 * inv[:, None, :]).astype(np.float32)                   # [L, F, F]
    b2f = (b2 * inv + beta - run_mean * inv).astype(np.float32)       # [L, F]

    # weights swizzled to [P, L, FT, F] (contraction chunk on partitions),
    # bf16 to halve the upload (adds ~2e-3 to the error budget)
    bf16 = ml_dtypes.bfloat16
    w1s = np.ascontiguousarray(
        W1.reshape(L, FT, P, F).transpose(2, 0, 1, 3).astype(bf16)
    )
    w2s = np.ascontiguousarray(
        w2f.reshape(L, FT, P, F).transpose(2, 0, 1, 3).astype(bf16)
    )
    # b1 as per-partition scalars: [P, L*FT]
    b1r = np.ascontiguousarray(
        b1.reshape(L, FT, P).transpose(2, 0, 1).reshape(P, L * FT)
    )
    # b2' (duplicated pair) on partition 0 only; the ones-matmul
    # broadcasts it into step3's PSUM
    b2r = np.zeros((P, L, 2 * F), dtype=bf16)
    b2r[0] = np.concatenate([b2f, b2f], axis=1).astype(bf16)
    ones_h = np.ones((P, P), dtype=bf16)

    if "nc" not in _cache:
        _cache["nc"] = _build_nc()
    nc = _cache["nc"]

    in_maps = []
    for c in range(NCORES):
        in_maps.append({
            "x0hi": np.ascontiguousarray(xhi8s[c * BPC:(c + 1) * BPC]),
            "bm": bm8,
            "w1": w1s,
            "w2": w2s,
            "b1": b1r,
            "b2": b2r,
            "ones": ones_h,
        })

    trace = os.environ.get("KERNEL_TRACE") == "1"
    res = run_bass_kernel_spmd(
        nc, in_maps, core_ids=list(range(NCORES)), trace=trace
    )
    _cache["last_results"] = res
    return np.concatenate([r["out"] for r in res.results], axis=1)


# revision 9
# speedup vs baseline: 1.2745x; 1.2745x over previous
"""GIN message-passing encoder (3 layers) on 8 Trainium2 NeuronCores.

Problem: x_{l+1} = relu(BN(relu((x + agg(x)) @ W1 + b1) @ W2 + b2)),
agg[b, d] = sum over edges (s -> d) of x[b, s]; output = stack of the 3
layer outputs, shape [3, 16, 1024, 256].

Strategy
--------
- Data parallel over batch: B=16 split as 2 batch elements per core.
- The scatter-add is a dense matmul against a host-built (N x N) matrix
  Bm[s, d] = I[s, d] + multiplicity(edge s -> d); the +x of GIN(eps=0)
  is the identity fold. Runs in fp8e4 DoubleRow (K=256/instruction).
- Eval-mode BatchNorm is folded into W2/b2 on the host; MLP matmuls in
  bf16.
- The PE stream is scheduled gapless: per layer the order is
  S1(b0) S1(b1) S2(b0) S2(b1) S3(b0) S3(b1), so each stage's producers
  (DVE/ACT casts and relus) complete during unrelated PE work. The PE
  p-state doubles its clock after ~3us of continuous work, so PE gaps
  cost double.
- Elementwise work is split between ACT and DVE so neither gates the
  PE: ACT does the ft0 m0-cast + half0 step2-relu + even-tp y-relu +
  odd-tp x-requant; DVE does the mirror set. (GPSIMD measured 3.6us
  per cast plus ~1us drains - unusable.)
- PSUM: step1 uses 2x [P,1024] bufs (4 banks); all step2/step3 groups
  share one 4-buf [P,512] pool (4 banks) so bank reuse never stalls.
- b2 bias enters step3's PSUM via a ones-matmul (lhsT=ones[128,128],
  rhs holds b2' pair on partition 0) in one N=512 instruction.
- Inputs are host-preswizzled; loads are spread across the scalar and
  vector queues in first-use order, stores ride the sync queue as
  [P,4,F] half-batches.
"""

import os

import numpy as np

BN_EPS = 1e-5

B, N, F = 16, 1024, 256
L = 3
NCORES = 8
BPC = B // NCORES  # batch elements per core
P = 128
NT = N // P   # 8 node tiles
FT = F // P   # 2 feature tiles
KK = N // 256  # 4 double-chunks of the contraction dim (DoubleRow K=256)
HALF = 512    # moving free-dim chunk
NH = N // HALF  # 2 halves of the node dim

_cache: dict = {}


def _build_nc():
    import concourse.bacc as bacc
    import concourse.mybir as mybir
    import concourse.tile as tile

    F32 = mybir.dt.float32
    F32R = mybir.dt.float32r
    F8 = mybir.dt.float8e4
    BF16 = mybir.dt.bfloat16
    Relu = mybir.ActivationFunctionType.Relu
    Copy = mybir.ActivationFunctionType.Copy
    Alu = mybir.AluOpType
    DR = mybir.MatmulPerfMode.DoubleRow

    nc = bacc.Bacc()

    x0hi_d = nc.dram_tensor("x0hi", [BPC, P, KK, 2, F], F8, kind="ExternalInput")
    bm_d = nc.dram_tensor("bm", [P, KK, 2, N], F8, kind="ExternalInput")
    w1_d = nc.dram_tensor("w1", [P, L, FT, F], BF16, kind="ExternalInput")
    w2_d = nc.dram_tensor("w2", [P, L, FT, F], BF16, kind="ExternalInput")
    b1_d = nc.dram_tensor("b1", [P, L * FT], F32, kind="ExternalInput")
    b2_d = nc.dram_tensor("b2", [P, L, 2 * F], BF16, kind="ExternalInput")
    ones_d = nc.dram_tensor("ones", [P, P], BF16, kind="ExternalInput")
    out_d = nc.dram_tensor("out", [L, BPC, N, F], F32R, kind="ExternalOutput")

    with tile.TileContext(nc) as tc:
        with (
            tc.tile_pool(name="const", bufs=1) as cpool,
            tc.tile_pool(name="x8", bufs=2) as xpool,
            tc.tile_pool(name="m0", bufs=2) as mpool,
            tc.tile_pool(name="h1", bufs=2) as hpool,
            tc.tile_pool(name="yt", bufs=4) as ypool,
            tc.tile_pool(name="pm0", bufs=2, space="PSUM") as pm0,
            tc.tile_pool(name="pmlp", bufs=4, space="PSUM") as pmlp,
        ):
            bm_sb = cpool.tile([P, KK, 2, N], F8)
            w1_sb = cpool.tile([P, L, FT, F], BF16)
            w2_sb = cpool.tile([P, L, FT, F], BF16)
            b1_sb = cpool.tile([P, L * FT], F32)
            b2z_sb = cpool.tile([P, L, 2 * F], BF16)
            ones_sb = cpool.tile([P, P], BF16)

            xhi = xpool.tile([P, BPC, KK, 2, F], F8, tag="xhi")

            # Input DMAs. All queues stripe over the same 16 DMA
            # engines, so order matters globally, not per queue: the
            # first matmul's pieces (bm kk0 + x(b0) kk0) go first and
            # alone, then the bulk in first-use order.
            nc.sync.dma_start(bm_sb[:, 0:1], bm_d[:, 0:1])
            nc.scalar.dma_start(xhi[:, 0, 0:1], x0hi_d[0, :, 0:1])
            for kk in range(1, KK):
                nc.sync.dma_start(bm_sb[:, kk:kk + 1], bm_d[:, kk:kk + 1])
            nc.scalar.dma_start(xhi[:, 0, 1:4], x0hi_d[0, :, 1:4])
            nc.scalar.dma_start(w1_sb[:], w1_d[:])
            nc.scalar.dma_start(b1_sb[:], b1_d[:])
            nc.sync.dma_start(xhi[:, 1], x0hi_d[1])
            nc.sync.dma_start(ones_sb[:], ones_d[:])
            nc.sync.dma_start(w2_sb[:], w2_d[:])
            nc.sync.dma_start(b2z_sb[:], b2_d[:])

            for l in range(L):
                last = l == L - 1
                if not last:
                    nxhi = xpool.tile([P, BPC, KK, 2, F], F8, tag="xhi")

                # ---- step 1: m0T = (A + I) @ x_q, fp8 DoubleRow ----
                # (l0, b0) runs kk-outermost: it consumes the still-
                # streaming bm at half the bandwidth (4 MMs per kk
                # chunk instead of 2). Steady state runs ft-outermost
                # so the psum group for ft0 completes early and its
                # cast overlaps the ft1 group's matmuls.
                m0t = [None, None]
                for b in range(BPC):
                    m0t[b] = mpool.tile([P, FT, N], BF16, tag="m0t", name=f"m0t{b}")
                    pss = [pm0.tile([P, NH * HALF], F32, tag="pm0", name=f"ps{ft}")
                           for ft in range(FT)]
                    cold = l == 0 and b == 0
                    order = (
                        [(kk, ft) for kk in range(KK) for ft in range(FT)]
                        if cold else
                        [(kk, ft) for ft in range(FT) for kk in range(KK)]
                    )
                    for kk, ft in order:
                        for half in range(NH):
                            nc.tensor.matmul(
                                pss[ft][:, half * HALF:(half + 1) * HALF],
                                xhi[:, b, kk, :, ft * P:(ft + 1) * P],
                                bm_sb[:, kk, :,
                                      half * HALF:(half + 1) * HALF],
                                start=(kk == 0),
                                stop=(kk == KK - 1),
                                perf_mode=DR,
                            )
                    # PSUM->SBUF cast split across ACT (ft0) and DVE
                    # (ft1): either alone is slower than the full-clock
                    # PE producing the next group.
                    nc.scalar.activation(m0t[b][:, 0, :], pss[0][:], Copy)
                    nc.vector.tensor_copy(m0t[b][:, 1, :], pss[1][:])

                h1t = [None, None]
                ysb = [[None, None], [None, None]]
                for b in range(BPC):
                    # ---- step 2: h1T = relu(W1^T-contract @ m0T + b1) ----
                    h1t[b] = hpool.tile([P, FT, N], BF16, tag="h1t", name=f"h1t{b}")
                    for gt in range(FT):
                        pg = [pmlp.tile([P, HALF], F32, tag="mlp", name=f"pg{h}")
                              for h in range(NH)]
                        for fk in range(FT):
                            for half in range(NH):
                                nc.tensor.matmul(
                                    pg[half][:],
                                    w1_sb[:, l, fk, gt * P:(gt + 1) * P],
                                    m0t[b][:, fk,
                                           half * HALF:(half + 1) * HALF],
                                    start=(fk == 0),
                                    stop=(fk == FT - 1),
                                )
                        # relu+bias split: ACT takes half0, DVE half1
                        nc.scalar.activation(
                            h1t[b][:, gt, 0:HALF], pg[0][:], Relu,
                            bias=b1_sb[:, l * FT + gt:l * FT + gt + 1],
                        )
                        nc.vector.tensor_scalar(
                            h1t[b][:, gt, HALF:N], pg[1][:],
                            b1_sb[:, l * FT + gt:l * FT + gt + 1], 0.0,
                            op0=Alu.add, op1=Alu.max,
                        )

                for b in range(BPC):
                    # ---- step 3: y = relu(h1 @ W2' + b2') -> out + next x ----
                    for j in range(2):        # output half: nodes [512j, 512j+512)
                        ysb[b][j] = ypool.tile([P, 4, F], F32R, tag="y", name=f"y{b}{j}")
                        for t2 in range(2):   # tp = 2j + t2
                            tp = 2 * j + t2
                            ps3 = pmlp.tile([P, 2, F], F32, tag="mlp")
                            # seed b2' into PSUM via a ones-matmul
                            # (N=512 covers both node tiles), then
                            # accumulate both tiles' GEMMs.
                            nc.tensor.matmul(
                                ps3[:], ones_sb[:], b2z_sb[:, l, :],
                                start=True, stop=False, skip_group_check=True,
                            )
                            for jj in range(2):
                                nt = 2 * tp + jj
                                for gk in range(FT):
                                    nc.tensor.matmul(
                                        ps3[:, jj, :],
                                        h1t[b][:, gk, nt * P:(nt + 1) * P],
                                        w2_sb[:, l, gk, :],
                                        start=False,
                                        stop=(gk == FT - 1),
                                        skip_group_check=True,
                                    )
                            ydst = ysb[b][j][:, 2 * t2:2 * t2 + 2, :]
                            if t2 == 0:
                                nc.scalar.activation(ydst, ps3[:], Relu)
                                if not last:
                                    nc.vector.tensor_scalar(
                                        nxhi[:, b, tp, :, :], ps3[:],
                                        0.0, None, op0=Alu.max,
                                    )
                            else:
                                nc.vector.tensor_scalar(
                                    ydst, ps3[:], 0.0, None, op0=Alu.max,
                                )
                                if not last:
                                    nc.scalar.activation(
                                        nxhi[:, b, tp, :, :], ps3[:], Relu,
                                    )
                        if last and b == BPC - 1:
                            # tail: split the final stores so the drain
                            # overlaps the remaining compute
                            for t2 in range(2):
                                nc.sync.dma_start(
                                    out_d[l, b,
                                          (4 * j + 2 * t2) * P:
                                          (4 * j + 2 * t2 + 2) * P, :]
                                    .rearrange("(t p) f -> p t f", p=P),
                                    ysb[b][j][:, 2 * t2:2 * t2 + 2, :],
                                )
                        else:
                            nc.sync.dma_start(
                                out_d[l, b, j * 4 * P:(j + 1) * 4 * P, :]
                                .rearrange("(t p) f -> p t f", p=P),
                                ysb[b][j][:],
                            )
                if not last:
                    xhi = nxhi

    nc.finalize()
    return nc


def kernel(h, edge_index, W1, b1, W2, b2, gamma, beta, run_mean, run_var):
    import ml_dtypes
    from concourse.bass_utils import run_bass_kernel_spmd

    f8 = ml_dtypes.float8_e4m3

    h = np.asarray(h, dtype=np.float32)
    edge_index = np.asarray(edge_index)
    W1 = np.asarray(W1, dtype=np.float32)
    b1 = np.asarray(b1, dtype=np.float32)
    W2 = np.asarray(W2, dtype=np.float32)
    b2 = np.asarray(b2, dtype=np.float32)
    gamma = np.asarray(gamma, dtype=np.float32)
    beta = np.asarray(beta, dtype=np.float32)
    run_mean = np.asarray(run_mean, dtype=np.float32)
    run_var = np.asarray(run_var, dtype=np.float32)

    # host-side preprocessing
    src = edge_index[0].astype(np.int64)
    dst = edge_index[1].astype(np.int64)
    bm = np.zeros((N, N), dtype=np.float32)
    np.add.at(bm, (src, dst), 1.0)
    bm[np.arange(N), np.arange(N)] += 1.0
    # fp8 exact for small integer counts; DoubleRow layout [P, KK, 2, N]
    bm8 = np.ascontiguousarray(
        bm.astype(f8).reshape(KK, 2, P, N).transpose(2, 0, 1, 3)
    )

    # x0 quantized to fp8 on the host, swizzled to [B, P, KK, 2, F]
    xhi8s = np.ascontiguousarray(
        h.astype(f8).reshape(B, KK, 2, P, F).transpose(0, 3, 1, 2, 4)
    )

    inv = (gamma / np.sqrt(run_var + BN_EPS)).astype(np.float32)      # [L, F]
    w2f = (W2 * inv[:, None, :]).astype(np.float32)                   # [L, F, F]
    b2f = (b2 * inv + beta - run_mean * inv).astype(np.float32)       # [L, F]

    # weights swizzled to [P, L, FT, F] (contraction chunk on partitions),
    # bf16 to halve the upload (adds ~2e-3 to the error budget)
    bf16 = ml_dtypes.bfloat16
    w1s = np.ascontiguousarray(
        W1.reshape(L, FT, P, F).transpose(2, 0, 1, 3).astype(bf16)
    )
    w2s = np.ascontiguousarray(
        w2f.reshape(L, FT, P, F).transpose(2, 0, 1, 3).astype(bf16)
    )
    # b1 as per-partition scalars: [P, L*FT]
    b1r = np.ascontiguousarray(
        b1.reshape(L, FT, P).transpose(2, 0, 1).reshape(P, L * FT)
    )
    # b2' (duplicated pair) on partition 0 only; the ones-matmul
    # broadcasts it into step3's PSUM
    b2r = np.zeros((P, L, 2 * F), dtype=bf16)
    b2r[0] = np.concatenate([b2f, b2f], axis=1).astype(bf16)
    ones_h = np.ones((P, P), dtype=bf16)

    if "nc" not in _cache:
        _cache["nc"] = _build_nc()
    nc = _cache["nc"]

    in_maps = []
    for c in range(NCORES):
        in_maps.append({
            "x0hi": np.ascontiguousarray(xhi8s[c * BPC:(c + 1) * BPC]),
            "bm": bm8,
            "w1": w1s,
            "w2": w2s,
            "b1": b1r,
            "b2": b2r,
            "ones": ones_h,
        })

    trace = os.environ.get("KERNEL_TRACE") == "1"
    res = run_bass_kernel_spmd(
        nc, in_maps, core_ids=list(range(NCORES)), trace=trace
    )
    _cache["last_results"] = res
    return np.concatenate([r["out"] for r in res.results], axis=1)
